# revision 25
# baseline (speedup 1.0000x reference)
"""Multi-head attention (MockCoreAttention) for 8 Trainium2 NeuronCores.

Problem: q,k,v [s=2048, b=2, n=16, d=128] fp32 ->
         out = softmax(q@k^T/sqrt(d)) @ v reshaped to [s, b, n*d].

Strategy (head parallel): 32 (b,n) heads sharded 4-per-core across 8 cores.
Per head, on-device:
  S^T[sk, sq] = K Q^T computed in 16 sk-chunks of 128 (TensorE, bf16 in,
  fp32 PSUM).  exp(S^T * scale) on ScalarE straight out of PSUM into SBUF
  (bf16) -- doubles as PSUM evacuation.  No max-subtraction pass: scores are
  ~N(0,1) (|max| < ~7), so fp32 exp cannot overflow and softmax is
  shift-invariant.
  O^T[d, sq] += V_j^T P_j^T accumulated over chunks in PSUM (TensorE).
  rowsum[q] accumulated the same way with an all-ones [128,128] stationary
  operand, which lands the sums row broadcast across all 128 partitions --
  so normalization is a plain elementwise multiply in O^T layout on DVE
  (reciprocal + tensor_tensor), no on-device transposes anywhere.
Host side does layout-only work: head sharding, [s,d]->[d,s] transposes,
bf16 cast, and the final gather/reshape.
"""

import numpy as np
import ml_dtypes

import sys

for _p in ("/opt/trn_rl_repo",):
    if _p not in sys.path:
        sys.path.append(_p)

S, B, NH, D = 2048, 2, 16, 128
H = B * NH            # 32 total heads
NCORES = 8
HL = H // NCORES      # 4 heads per core
SOFTMAX_SCALE = 0.08838834764831845  # 1/sqrt(128)

BF16 = ml_dtypes.bfloat16

# Schraudolph exp-in-bf16 for the DVE offload path: exp(s*SCALE) ~=
# bitcast_bf16(int16(s*A + B)).  B tuned on HW for zero mean relative
# error (rms ~1.8%); scores are N(0, sqrt(d)) so s*A + B stays in
# [~15200, ~17300], well inside int16.
DVE_EXP_A = SOFTMAX_SCALE * 128.0 / 0.6931471805599453  # = SCALE*2^7/ln2
DVE_EXP_B = 16248.6


def build_program(s=S, hl=HL, sq=1024, nmm=512, mm_dtype="bf16", repeat=1,
                  stages=("qk", "exp", "pv", "sums", "tail"), lookahead=1,
                  s_bufs=2, pt_bufs=4, o_bufs=1, sm_bufs=1, exp_fuse=1,
                  interleave_groups=False, sum_fold=1, prefetch=False,
                  tail_bufs=2, fs_bufs=None, in_bufs=2, bench_internal=False,
                  tail_defer=0, recip_fast=True, sum_merge=False,
                  pool_adds=0, dve_exp_period=0, dve_exp_phase=3,
                  band_mode=False, band_w=2):
    """Build the per-core Bass/Tile program (SPMD: identical on all cores).

    s: sequence length, hl: heads per core, sq: q-columns per group
    (PSUM budget: 2*sq (S^T double buf) + sq (O^T) + sq (sums) <= 4096 fp32),
    nmm: moving free-dim per matmul instruction.
    """
    import concourse.tile as tile
    from concourse import bacc, mybir

    j_chunks = s // 128
    groups = s // sq
    assert sq % nmm == 0
    nsub = sq // nmm
    assert j_chunks % exp_fuse == 0
    jj_units = j_chunks // exp_fuse
    # sum_fold=0: chain-accumulate all chunks on DVE, single transient
    # sums matmul per group allocated from the s-pool (frees the sums bank).
    # sum_fold=-1: like 0, but the final cross-partition reduce runs on the
    # otherwise-idle GpSimd (partition_all_reduce) — no PE matmul, no PSUM.
    # sum_fold=-2: chain adds + transient ones-matmul in a DEDICATED sums
    # bank (smp pool) instead of borrowing an s-pool slot.
    assert sum_fold in (-2, -1, 0, 1, 2, 4) and (sum_fold <= 0 or
                                                 j_chunks % sum_fold == 0)
    assert sum_fold in (-2, -1, 0, 1) or exp_fuse == 1

    dt_in = {"bf16": mybir.dt.bfloat16, "fp32r": mybir.dt.float32r}[mm_dtype]
    np_in = {"bf16": BF16, "fp32r": np.float32}[mm_dtype]

    nc = bacc.Bacc("TRN2", target_bir_lowering=False, debug=False,
                   enable_asserts=False)

    # bench_internal: big tensors live in device DRAM (Internal) and are
    # initialized on-device, so each benchmark call ships only a dummy
    # [2,1] in/out over the (noisy, slow) axon link. Timing-identical
    # device loop, ~10x less wall-clock noise.
    kind_in = "Internal" if bench_internal else "ExternalInput"
    kind_out = "Internal" if bench_internal else "ExternalOutput"
    qt = nc.dram_tensor("qt", [hl, D, s], dt_in, kind=kind_in).ap()
    kt = nc.dram_tensor("kt", [hl, D, s], dt_in, kind=kind_in).ap()
    vp = nc.dram_tensor("vp", [hl, 128, j_chunks, D], dt_in,
                        kind=kind_in).ap()
    outT = nc.dram_tensor("outT", [hl, D, s], mybir.dt.float32,
                          kind=kind_out).ap()
    if bench_internal:
        dummy_in = nc.dram_tensor("dummy_in", [2, 1], mybir.dt.float32,
                                  kind="ExternalInput").ap()
        dummy_out = nc.dram_tensor("dummy_out", [2, 1], mybir.dt.float32,
                                   kind="ExternalOutput").ap()

    f32 = mybir.dt.float32
    Exp = mybir.ActivationFunctionType.Exp

    with tile.TileContext(nc) as tc:
        with (
            tc.tile_pool(name="singles", bufs=1) as singles,
            tc.tile_pool(name="qin", bufs=hl if prefetch else in_bufs) as qin,
            tc.tile_pool(name="kin", bufs=hl if prefetch else in_bufs) as kin,
            tc.tile_pool(name="vin", bufs=hl if prefetch else in_bufs) as vin,
            tc.tile_pool(name="pt", bufs=pt_bufs) as ptp,
            tc.tile_pool(name="spsum", bufs=s_bufs, space="PSUM") as sp,
            tc.tile_pool(name="opsum", bufs=o_bufs, space="PSUM") as op,
            tc.tile_pool(name="smpsum", bufs=sm_bufs, space="PSUM") as smp,
            tc.tile_pool(name="outsb", bufs=tail_bufs) as outsb,
            tc.tile_pool(name="recip", bufs=tail_bufs) as rcp,
            tc.tile_pool(name="fsum",
                         bufs=fs_bufs or max(3, 2 * sum_fold)) as fsp,
        ):
            ones = singles.tile([128, 128], dt_in)
            nc.vector.memset(ones[:], 1.0)

            if bench_internal:
                # init the Internal qt/kt/vp with a finite constant so the
                # timed loop sees realistic (non-garbage) data
                seed = singles.tile([128, s], dt_in, name="seed")
                nc.vector.memset(seed[:], 0.125)
                seedv = singles.tile([128, j_chunks, D], dt_in, name="seedv")
                nc.vector.memset(seedv[:], 0.125)
                for h in range(hl):
                    nc.sync.dma_start(qt[h, :, :], seed[:])
                    nc.sync.dma_start(kt[h, :, :], seed[:])
                    nc.sync.dma_start(vp[h, :, :, :], seedv[:])

            def body(_it=None):
                head_tiles = {}
                group_psum = {}
                fold_state = {}
                chunk_counter = [0]

                def ensure_head(h):
                    if h not in head_tiles:
                        qt_t = qin.tile([D, s], dt_in)
                        nc.sync.dma_start(qt_t[:], qt[h, :, :])
                        kt_t = kin.tile([D, s], dt_in)
                        nc.sync.dma_start(kt_t[:], kt[h, :, :])
                        vp_t = vin.tile([128, j_chunks, D], dt_in)
                        nc.sync.dma_start(vp_t[:], vp[h, :, :, :])
                        head_tiles[h] = (qt_t, kt_t, vp_t)
                    return head_tiles[h]

                def emit_qk(h, g, jj):
                    qt_t, kt_t, _ = ensure_head(h)
                    s_t = sp.tile([128, exp_fuse, sq], f32)
                    if "qk" in stages:
                        for u in range(exp_fuse):
                            j = jj * exp_fuse + u
                            for c in range(nsub):
                                nc.tensor.matmul(
                                    s_t[:, u, c * nmm:(c + 1) * nmm],
                                    lhsT=kt_t[:, j * 128:(j + 1) * 128],
                                    rhs=qt_t[:, g * sq + c * nmm:
                                             g * sq + (c + 1) * nmm],
                                )
                    return s_t

                def emit_rest(h, g, jj, s_t):
                    _, _, vp_t = head_tiles[h]
                    if (h, g) not in group_psum:
                        group_psum[(h, g)] = (
                            op.tile([D, sq], f32, name="o_t"),
                            None if sum_fold <= 0 else
                            smp.tile([128, sq], f32, name="sm_t"))
                    o_t, sm_t = group_psum[(h, g)]
                    chunk_no = chunk_counter[0]
                    chunk_counter[0] += 1
                    use_dve = (dve_exp_period and
                               chunk_no % dve_exp_period == dve_exp_phase)
                    if use_dve:
                        # Schraudolph exp on DVE: bitcast(int16(x*A + B)) in
                        # bf16 domain -- one tensor_scalar, ~1.8% rms error,
                        # offloads the ScalarE exp bottleneck
                        pt_i = ptp.tile([128, exp_fuse, sq], mybir.dt.int16,
                                        name="pt")
                        if "exp" in stages:
                            nc.vector.tensor_scalar(
                                pt_i[:], s_t[:], DVE_EXP_A, DVE_EXP_B,
                                op0=mybir.AluOpType.mult,
                                op1=mybir.AluOpType.add)
                        pt_t = pt_i.bitcast(mybir.dt.bfloat16)
                    else:
                        pt_t = ptp.tile([128, exp_fuse, sq], dt_in,
                                        name="pt")
                        if "exp" in stages:
                            nc.scalar.activation(pt_t[:], s_t[:], Exp,
                                                 scale=SOFTMAX_SCALE)
                    if sum_merge and "sums" in stages:
                        # u-merged chain: one [128, exp_fuse*sq]-wide add per
                        # chunk-pair (half the DVE instructions); first
                        # pool_adds adds go to the idle GpSimd/Pool engine
                        if jj == 0:
                            fold_state[(h, g)] = pt_t[:]
                        else:
                            acc = fold_state[(h, g)]
                            t = fsp.tile([128, exp_fuse, sq], dt_in,
                                         name="fs")
                            eng = (nc.gpsimd if jj <= pool_adds
                                   else nc.vector)
                            eng.tensor_add(t[:], acc, pt_t[:])
                            fold_state[(h, g)] = t[:]
                        if (jj + 1) * exp_fuse == j_chunks:
                            acc = fold_state.pop((h, g))
                            mrg = fsp.tile([128, sq], dt_in, name="fsm")
                            nc.vector.tensor_add(mrg[:], acc[:, 0, :],
                                                 acc[:, 1, :])
                            if sum_fold == -2:
                                sm_t = smp.tile([128, sq], f32, name="sm_t")
                                smdst = sm_t[:]
                            else:
                                sm_t = sp.tile([128, exp_fuse, sq], f32,
                                               name="sm_t", tag="s_t")
                                smdst = sm_t[:, 0, :]
                            for c in range(nsub):
                                cs = slice(c * nmm, (c + 1) * nmm)
                                nc.tensor.matmul(
                                    smdst[:, cs], lhsT=ones[:],
                                    rhs=mrg[:, cs], start=True, stop=True)
                            group_psum[(h, g)] = (o_t, smdst)
                    for u in range(exp_fuse):
                        j = jj * exp_fuse + u
                        first, last = (j == 0), (j == j_chunks - 1)
                        if "pv" in stages:
                            for c in range(nsub):
                                cs = slice(c * nmm, (c + 1) * nmm)
                                nc.tensor.matmul(
                                    o_t[:, cs], lhsT=vp_t[:, j, :],
                                    rhs=pt_t[:, u, cs],
                                    start=first, stop=last)
                        if "sums" in stages and not sum_merge:
                            if sum_fold <= 0:
                                if j == 0:
                                    fold_state[(h, g)] = pt_t[:, u, :]
                                else:
                                    acc = fold_state[(h, g)]
                                    t = fsp.tile([128, sq], dt_in, name="fs")
                                    nc.vector.tensor_add(t[:], acc,
                                                         pt_t[:, u, :])
                                    fold_state[(h, g)] = t[:]
                                if last and sum_fold == 0:
                                    fin = fold_state.pop((h, g))
                                    sm_t = sp.tile([128, exp_fuse, sq], f32,
                                                   name="sm_t", tag="s_t")
                                    for c in range(nsub):
                                        cs = slice(c * nmm, (c + 1) * nmm)
                                        nc.tensor.matmul(
                                            sm_t[:, 0, cs], lhsT=ones[:],
                                            rhs=fin[:, cs],
                                            start=True, stop=True)
                                    group_psum[(h, g)] = (o_t, sm_t[:, 0, :])
                                elif last and sum_fold == -2:
                                    fin = fold_state.pop((h, g))
                                    sm_t = smp.tile([128, sq], f32,
                                                    name="sm_t")
                                    for c in range(nsub):
                                        cs = slice(c * nmm, (c + 1) * nmm)
                                        nc.tensor.matmul(
                                            sm_t[:, cs], lhsT=ones[:],
                                            rhs=fin[:, cs],
                                            start=True, stop=True)
                                    group_psum[(h, g)] = (o_t, sm_t[:])
                                elif last:
                                    from concourse import bass_isa
                                    smg = fsp.tile([128, sq], f32, name="smg")
                                    nc.gpsimd.partition_all_reduce(
                                        smg[:], fold_state.pop((h, g)),
                                        channels=128,
                                        reduce_op=bass_isa.ReduceOp.add)
                                    group_psum[(h, g)] = (o_t, smg[:])
                            elif sum_fold == 1:
                                for c in range(nsub):
                                    cs = slice(c * nmm, (c + 1) * nmm)
                                    nc.tensor.matmul(
                                        sm_t[:, cs], lhsT=ones[:],
                                        rhs=pt_t[:, u, cs],
                                        start=first, stop=last)
                            else:
                                # Fold pairs/quads of P^T chunks on DVE (bf16
                                # 2x) so the ones-matmul streams fewer columns
                                # on PE.  fp32 PSUM still does the final
                                # accumulation across fold-runs.
                                fl = fold_state.setdefault((h, g), [])
                                fl.append(pt_t[:, u, :])
                                if len(fl) == sum_fold:
                                    work = list(fl)
                                    fl.clear()
                                    while len(work) > 1:
                                        nxt = []
                                        for a, b in zip(work[::2], work[1::2]):
                                            t = fsp.tile([128, sq], dt_in,
                                                         name="fs")
                                            nc.vector.tensor_add(t[:], a, b)
                                            nxt.append(t[:])
                                        work = nxt
                                    run = j // sum_fold
                                    for c in range(nsub):
                                        cs = slice(c * nmm, (c + 1) * nmm)
                                        nc.tensor.matmul(
                                            sm_t[:, cs], lhsT=ones[:],
                                            rhs=work[0][:, cs],
                                            start=(run == 0),
                                            stop=(run == j_chunks //
                                                  sum_fold - 1))
                    return (h, g) if (jj + 1) * exp_fuse == j_chunks else None

                def emit_tail(h, g):
                    has_tail = ("tail" in stages or
                                any(s.startswith("tail_") for s in stages))
                    if not has_tail:
                        group_psum.pop((h, g), None)
                        return
                    o_t, sm_t = group_psum[(h, g)]
                    full_tail = "tail" in stages
                    recip_t = rcp.tile([128, sq], f32)
                    if recip_fast:
                        # ~51 ULP, ~5x faster than InstReciprocal; inputs are
                        # sums of positive exps (1e2..1e5), no edge cases
                        nc.vector.reciprocal_approx_fast(recip_t[:], sm_t[:])
                    else:
                        nc.vector.reciprocal(recip_t[:], sm_t[:])
                    if full_tail or "tail_mul" in stages \
                            or "tail_dma" in stages:
                        otn = outsb.tile([D, sq], f32)
                        nc.vector.tensor_mul(otn[:], o_t[:], recip_t[:])
                        if full_tail or "tail_dma" in stages:
                            nc.sync.dma_start(
                                outT[h, :, g * sq:(g + 1) * sq], otn[:])
                    del group_psum[(h, g)]

                if prefetch:
                    for h in range(hl):
                        ensure_head(h)
                if interleave_groups and groups >= 2:
                    chunk_list = [(h, p + g, jj) for h in range(hl)
                                  for p in range(0, groups, 2)
                                  for jj in range(jj_units)
                                  for g in range(min(2, groups - p))]
                else:
                    chunk_list = [(h, g, jj) for h in range(hl)
                                  for g in range(groups)
                                  for jj in range(jj_units)]
                pending = []
                tails = []  # [(h, g, countdown)] deferred group tails

                def step_tails(flush=False):
                    while tails and (flush or tails[0][2] <= 0):
                        th, tg, _ = tails.pop(0)
                        emit_tail(th, tg)
                    for i_ in range(len(tails)):
                        th, tg, cd = tails[i_]
                        tails[i_] = (th, tg, cd - 1)

                for ch in chunk_list:
                    pending.append((ch, emit_qk(*ch)))
                    if len(pending) > lookahead:
                        (h, g, j), s_t = pending.pop(0)
                        done = emit_rest(h, g, j, s_t)
                        if done:
                            tails.append((*done, tail_defer))
                        step_tails()
                while pending:
                    (h, g, j), s_t = pending.pop(0)
                    done = emit_rest(h, g, j, s_t)
                    if done:
                        tails.append((*done, tail_defer))
                    step_tails()
                step_tails(flush=True)

            def body_band(_it=None):
                """Band mode: one s/pt tile holds band_w groups' columns for
                the SAME k-chunk j, so each kt[j]/vp[j] weight load serves
                band_w matmuls (PE per exp-unit drops below the ScalarE exp
                time, breaking the PE<->ScalarE lockstep).  Iteration order:
                (head, band, j)."""
                bw = band_w
                nb = groups // bw
                assert groups % bw == 0
                head_tiles = {}
                o_tiles = {}
                band_acc = {}
                group_psum = {}
                unit_counter = [0]

                def ensure_head(h):
                    if h not in head_tiles:
                        qt_t = qin.tile([D, s], dt_in)
                        nc.sync.dma_start(qt_t[:], qt[h, :, :])
                        kt_t = kin.tile([D, s], dt_in)
                        nc.sync.dma_start(kt_t[:], kt[h, :, :])
                        vp_t = vin.tile([128, j_chunks, D], dt_in)
                        nc.sync.dma_start(vp_t[:], vp[h, :, :, :])
                        head_tiles[h] = (qt_t, kt_t, vp_t)
                    return head_tiles[h]

                def emit_qk_band(h, band, j):
                    qt_t, kt_t, _ = ensure_head(h)
                    s_bt = sp.tile([128, bw, sq], f32)
                    if "qk" in stages:
                        for gb in range(bw):
                            g = band * bw + gb
                            for c in range(nsub):
                                cs = slice(c * nmm, (c + 1) * nmm)
                                nc.tensor.matmul(
                                    s_bt[:, gb, cs],
                                    lhsT=kt_t[:, j * 128:(j + 1) * 128],
                                    rhs=qt_t[:, g * sq + c * nmm:
                                             g * sq + (c + 1) * nmm])
                    return s_bt

                def emit_rest_band(h, band, j, s_bt):
                    _, _, vp_t = head_tiles[h]
                    uno = unit_counter[0]
                    unit_counter[0] += 1
                    use_dve = (dve_exp_period and
                               uno % dve_exp_period == dve_exp_phase)
                    if use_dve:
                        pt_i = ptp.tile([128, bw, sq], mybir.dt.int16,
                                        name="pt")
                        if "exp" in stages:
                            nc.vector.tensor_scalar(
                                pt_i[:], s_bt[:], DVE_EXP_A, DVE_EXP_B,
                                op0=mybir.AluOpType.mult,
                                op1=mybir.AluOpType.add)
                        pt_bt = pt_i.bitcast(mybir.dt.bfloat16)
                    else:
                        pt_bt = ptp.tile([128, bw, sq], dt_in, name="pt")
                        if "exp" in stages:
                            nc.scalar.activation(pt_bt[:], s_bt[:], Exp,
                                                 scale=SOFTMAX_SCALE)
                    first, last = (j == 0), (j == j_chunks - 1)
                    if "pv" in stages:
                        for gb in range(bw):
                            g = band * bw + gb
                            if (h, g) not in o_tiles:
                                o_tiles[(h, g)] = op.tile([D, sq], f32,
                                                          name="o_t")
                            for c in range(nsub):
                                cs = slice(c * nmm, (c + 1) * nmm)
                                nc.tensor.matmul(
                                    o_tiles[(h, g)][:, cs],
                                    lhsT=vp_t[:, j, :],
                                    rhs=pt_bt[:, gb, cs],
                                    start=first, stop=last)
                    if "sums" in stages:
                        if first:
                            band_acc[(h, band)] = pt_bt[:]
                        else:
                            t = fsp.tile([128, bw, sq], dt_in, name="fs")
                            nc.vector.tensor_add(t[:], band_acc[(h, band)],
                                                 pt_bt[:])
                            band_acc[(h, band)] = t[:]
                        if last:
                            acc = band_acc.pop((h, band))
                            if sum_fold == -2:
                                sm_t = smp.tile([128, bw, sq], f32,
                                                name="sm_t")
                                sm_aps = [sm_t[:, gb, :] for gb in range(bw)]
                            else:
                                sm_t = sp.tile([128, bw, sq], f32,
                                               name="sm_t", tag="s_t")
                                sm_aps = [sm_t[:, gb, :] for gb in range(bw)]
                            for gb in range(bw):
                                g = band * bw + gb
                                for c in range(nsub):
                                    cs = slice(c * nmm, (c + 1) * nmm)
                                    nc.tensor.matmul(
                                        sm_aps[gb][:, cs], lhsT=ones[:],
                                        rhs=acc[:, gb, cs],
                                        start=True, stop=True)
                                group_psum[(h, g)] = (
                                    o_tiles.pop((h, g), None), sm_aps[gb])
                    return ([(h, band * bw + gb) for gb in range(bw)]
                            if last else None)

                def emit_tail_band(h, g):
                    if "tail" not in stages:
                        group_psum.pop((h, g), None)
                        return
                    o_t, sm_ap = group_psum[(h, g)]
                    recip_t = rcp.tile([128, sq], f32)
                    if recip_fast:
                        nc.vector.reciprocal_approx_fast(recip_t[:], sm_ap)
                    else:
                        nc.vector.reciprocal(recip_t[:], sm_ap)
                    otn = outsb.tile([D, sq], f32)
                    nc.vector.tensor_mul(otn[:], o_t[:], recip_t[:])
                    nc.sync.dma_start(outT[h, :, g * sq:(g + 1) * sq],
                                      otn[:])
                    del group_psum[(h, g)]

                if prefetch:
                    for h in range(hl):
                        ensure_head(h)
                unit_list = [(h, b, j) for h in range(hl)
                             for b in range(nb) for j in range(j_chunks)]
                pending = []
                tails = []

                def step_tails(flush=False):
                    while tails and (flush or tails[0][2] <= 0):
                        th, tg, _ = tails.pop(0)
                        emit_tail_band(th, tg)
                    for i_ in range(len(tails)):
                        th, tg, cd = tails[i_]
                        tails[i_] = (th, tg, cd - 1)

                for unit in unit_list:
                    pending.append((unit, emit_qk_band(*unit)))
                    if len(pending) > lookahead:
                        u, s_bt = pending.pop(0)
                        done = emit_rest_band(*u, s_bt)
                        if done:
                            tails.extend((hh, gg, tail_defer)
                                         for (hh, gg) in done)
                        step_tails()
                while pending:
                    u, s_bt = pending.pop(0)
                    done = emit_rest_band(*u, s_bt)
                    if done:
                        tails.extend((hh, gg, tail_defer)
                                     for (hh, gg) in done)
                    step_tails()
                step_tails(flush=True)

            if band_mode:
                body = body_band

            if repeat == 1:
                body()
            elif repeat < 0:
                # negative: Python-unrolled (for TimelineSim, which can't
                # follow For_i register branches without an executor)
                for _r in range(-repeat):
                    body(_r)
            else:
                with tc.For_i(0, repeat, 1) as _i:
                    body(_i)

            if bench_internal:
                dtile = singles.tile([2, 1], mybir.dt.float32, name="dumt")
                nc.sync.dma_start(dtile[:], dummy_in[:])
                nc.sync.dma_start(dummy_out[:], dtile[:])

    nc.compile()
    return nc, np_in


def shard_inputs(q, k, v, s=S, hl=HL, ncores=NCORES, np_in=BF16):
    """Host-side layout prep: per-core per-head transposed views, cast."""
    nheads = ncores * hl
    j_chunks = s // 128
    # [s,b,n,d] -> [b,n,d,s] -> [H, d, s]
    qt = np.ascontiguousarray(q.transpose(1, 2, 3, 0)).reshape(nheads, D, s)
    kt = np.ascontiguousarray(k.transpose(1, 2, 3, 0)).reshape(nheads, D, s)
    # [s,b,n,d] -> [b,n,s,d] -> [H, J, 128, d] -> [H, 128, J, d]
    vpm = (v.transpose(1, 2, 0, 3)
            .reshape(nheads, j_chunks, 128, D)
            .transpose(0, 2, 1, 3))
    qt = qt.astype(np_in)
    kt = kt.astype(np_in)
    vpm = np.ascontiguousarray(vpm).astype(np_in)
    in_maps = []
    for c in range(ncores):
        sl = slice(c * hl, (c + 1) * hl)
        in_maps.append({
            "qt": np.ascontiguousarray(qt[sl]),
            "kt": np.ascontiguousarray(kt[sl]),
            "vp": np.ascontiguousarray(vpm[sl]),
        })
    return in_maps


def gather_output(results, s=S, hl=HL, ncores=NCORES):
    """[{outT: [hl, D, s]}] per core -> full [s, B, NH*D] fp32."""
    outT = np.stack([np.asarray(r["outT"]) for r in results])  # [C, hl, D, s]
    out_heads = outT.reshape(ncores * hl, D, s)                # [H, D, s]
    out = out_heads.transpose(2, 0, 1)                         # [s, H, D]
    return np.ascontiguousarray(out).reshape(s, B, NH * D)


_CACHE = {}

# Best measured configuration (~145 us/core steady state, vs ~138 us ScalarE
# exp roofline):
#   recip_fast     -- InstReciprocal is ~5x a normal DVE op and was blocking
#                     the in-order DVE queue (+45 us); approx_fast fixes it.
#   sum_merge      -- one u-merged [128,1024] chain add per chunk pair
#                     (half the DVE sums instructions).
#   sum_fold=-2    -- sums ones-matmul lands in a dedicated double-buffered
#                     PSUM bank (sm_bufs=2) instead of borrowing a QK<->exp
#                     s-pool slot; PSUM: s 2x2 + sm 2 + o 2 = 8 banks.
#   in_bufs=3      -- triple-buffered per-head input DMAs.
BEST_CFG = dict(sq=512, nmm=512, mm_dtype="bf16", lookahead=2, s_bufs=2,
                o_bufs=2, sm_bufs=2, exp_fuse=2, sum_fold=-2, sum_merge=True,
                in_bufs=3, pt_bufs=8, tail_bufs=3, fs_bufs=5,
                recip_fast=True)


def _get_program(**cfg):
    key = tuple(sorted(cfg.items()))
    if key not in _CACHE:
        _CACHE[key] = build_program(**cfg)
    return _CACHE[key]


def run(q, k, v, trace=False, **cfg):
    """Run on the 8 NeuronCores; returns (out, BassKernelResults)."""
    from concourse.bass_utils import run_bass_kernel_spmd

    full_cfg = {**BEST_CFG, **cfg}
    nc, np_in = _get_program(**full_cfg)
    in_maps = shard_inputs(q, k, v, np_in=np_in)
    res = run_bass_kernel_spmd(nc, in_maps, core_ids=list(range(NCORES)),
                               trace=trace)
    return gather_output(res.results), res


def kernel(q, k, v):
    q = np.asarray(q, dtype=np.float32)
    k = np.asarray(k, dtype=np.float32)
    v = np.asarray(v, dtype=np.float32)
    out, _ = run(q, k, v)
    return out



# revision 29
# speedup vs baseline: 1.0261x; 1.0261x over previous
"""Multi-head attention (MockCoreAttention) for 8 Trainium2 NeuronCores.

Problem: q,k,v [s=2048, b=2, n=16, d=128] fp32 ->
         out = softmax(q@k^T/sqrt(d)) @ v reshaped to [s, b, n*d].

Strategy (head parallel): 32 (b,n) heads sharded 4-per-core across 8 cores.
Per head, on-device:
  S^T[sk, sq] = K Q^T computed in 16 sk-chunks of 128 (TensorE, bf16 in,
  fp32 PSUM).  exp(S^T * scale) on ScalarE straight out of PSUM into SBUF
  (bf16) -- doubles as PSUM evacuation.  No max-subtraction pass: scores are
  ~N(0,1) (|max| < ~7), so fp32 exp cannot overflow and softmax is
  shift-invariant.
  O^T[d, sq] += V_j^T P_j^T accumulated over chunks in PSUM (TensorE).
  rowsum[q] accumulated the same way with an all-ones [128,128] stationary
  operand, which lands the sums row broadcast across all 128 partitions --
  so normalization is a plain elementwise multiply in O^T layout on DVE
  (reciprocal + tensor_tensor), no on-device transposes anywhere.
Host side does layout-only work: head sharding, [s,d]->[d,s] transposes,
bf16 cast, and the final gather/reshape.
"""

import numpy as np
import ml_dtypes

import sys

for _p in ("/opt/trn_rl_repo",):
    if _p not in sys.path:
        sys.path.append(_p)

S, B, NH, D = 2048, 2, 16, 128
H = B * NH            # 32 total heads
NCORES = 8
HL = H // NCORES      # 4 heads per core
SOFTMAX_SCALE = 0.08838834764831845  # 1/sqrt(128)

BF16 = ml_dtypes.bfloat16

# Schraudolph exp-in-bf16 for the DVE offload path: exp(s*SCALE) ~=
# bitcast_bf16(int16(s*A + B)).  B tuned on HW for zero mean relative
# error (rms ~1.8%); scores are N(0, sqrt(d)) so s*A + B stays in
# [~15200, ~17300], well inside int16.
DVE_EXP_A = SOFTMAX_SCALE * 128.0 / 0.6931471805599453  # = SCALE*2^7/ln2
DVE_EXP_B = 16248.6


def build_program(s=S, hl=HL, sq=1024, nmm=512, mm_dtype="bf16", repeat=1,
                  stages=("qk", "exp", "pv", "sums", "tail"), lookahead=1,
                  s_bufs=2, pt_bufs=4, o_bufs=1, sm_bufs=1, exp_fuse=1,
                  interleave_groups=False, sum_fold=1, prefetch=False,
                  tail_bufs=2, fs_bufs=None, in_bufs=2, bench_internal=False,
                  tail_defer=0, recip_fast=True, sum_merge=False,
                  pool_adds=0, dve_exp_period=0, dve_exp_phase=3,
                  band_mode=False, band_w=2, sm_defer=False):
    """Build the per-core Bass/Tile program (SPMD: identical on all cores).

    s: sequence length, hl: heads per core, sq: q-columns per group
    (PSUM budget: 2*sq (S^T double buf) + sq (O^T) + sq (sums) <= 4096 fp32),
    nmm: moving free-dim per matmul instruction.
    """
    import concourse.tile as tile
    from concourse import bacc, mybir

    j_chunks = s // 128
    groups = s // sq
    assert sq % nmm == 0
    nsub = sq // nmm
    assert j_chunks % exp_fuse == 0
    jj_units = j_chunks // exp_fuse
    # sum_fold=0: chain-accumulate all chunks on DVE, single transient
    # sums matmul per group allocated from the s-pool (frees the sums bank).
    # sum_fold=-1: like 0, but the final cross-partition reduce runs on the
    # otherwise-idle GpSimd (partition_all_reduce) — no PE matmul, no PSUM.
    # sum_fold=-2: chain adds + transient ones-matmul in a DEDICATED sums
    # bank (smp pool) instead of borrowing an s-pool slot.
    assert sum_fold in (-2, -1, 0, 1, 2, 4) and (sum_fold <= 0 or
                                                 j_chunks % sum_fold == 0)
    assert sum_fold in (-2, -1, 0, 1) or exp_fuse == 1

    dt_in = {"bf16": mybir.dt.bfloat16, "fp32r": mybir.dt.float32r}[mm_dtype]
    np_in = {"bf16": BF16, "fp32r": np.float32}[mm_dtype]

    nc = bacc.Bacc("TRN2", target_bir_lowering=False, debug=False,
                   enable_asserts=False)

    # bench_internal: big tensors live in device DRAM (Internal) and are
    # initialized on-device, so each benchmark call ships only a dummy
    # [2,1] in/out over the (noisy, slow) axon link. Timing-identical
    # device loop, ~10x less wall-clock noise.
    kind_in = "Internal" if bench_internal else "ExternalInput"
    kind_out = "Internal" if bench_internal else "ExternalOutput"
    qt = nc.dram_tensor("qt", [hl, D, s], dt_in, kind=kind_in).ap()
    kt = nc.dram_tensor("kt", [hl, D, s], dt_in, kind=kind_in).ap()
    vp = nc.dram_tensor("vp", [hl, 128, j_chunks, D], dt_in,
                        kind=kind_in).ap()
    outT = nc.dram_tensor("outT", [hl, D, s], mybir.dt.float32,
                          kind=kind_out).ap()
    if bench_internal:
        dummy_in = nc.dram_tensor("dummy_in", [2, 1], mybir.dt.float32,
                                  kind="ExternalInput").ap()
        dummy_out = nc.dram_tensor("dummy_out", [2, 1], mybir.dt.float32,
                                   kind="ExternalOutput").ap()

    f32 = mybir.dt.float32
    Exp = mybir.ActivationFunctionType.Exp

    with tile.TileContext(nc) as tc:
        with (
            tc.tile_pool(name="singles", bufs=1) as singles,
            tc.tile_pool(name="qin", bufs=hl if prefetch else in_bufs) as qin,
            tc.tile_pool(name="kin", bufs=hl if prefetch else in_bufs) as kin,
            tc.tile_pool(name="vin", bufs=hl if prefetch else in_bufs) as vin,
            tc.tile_pool(name="pt", bufs=pt_bufs) as ptp,
            tc.tile_pool(name="spsum", bufs=s_bufs, space="PSUM") as sp,
            tc.tile_pool(name="opsum", bufs=o_bufs, space="PSUM") as op,
            tc.tile_pool(name="smpsum", bufs=sm_bufs, space="PSUM") as smp,
            tc.tile_pool(name="outsb", bufs=tail_bufs) as outsb,
            tc.tile_pool(name="recip", bufs=tail_bufs) as rcp,
            tc.tile_pool(name="fsum",
                         bufs=fs_bufs or max(3, 2 * sum_fold)) as fsp,
        ):
            ones = singles.tile([128, 128], dt_in)
            nc.vector.memset(ones[:], 1.0)

            if bench_internal:
                # init the Internal qt/kt/vp with a finite constant so the
                # timed loop sees realistic (non-garbage) data
                seed = singles.tile([128, s], dt_in, name="seed")
                nc.vector.memset(seed[:], 0.125)
                seedv = singles.tile([128, j_chunks, D], dt_in, name="seedv")
                nc.vector.memset(seedv[:], 0.125)
                for h in range(hl):
                    nc.sync.dma_start(qt[h, :, :], seed[:])
                    nc.sync.dma_start(kt[h, :, :], seed[:])
                    nc.sync.dma_start(vp[h, :, :, :], seedv[:])

            def body(_it=None):
                head_tiles = {}
                group_psum = {}
                fold_state = {}
                chunk_counter = [0]

                def ensure_head(h):
                    if h not in head_tiles:
                        qt_t = qin.tile([D, s], dt_in)
                        nc.sync.dma_start(qt_t[:], qt[h, :, :])
                        kt_t = kin.tile([D, s], dt_in)
                        nc.sync.dma_start(kt_t[:], kt[h, :, :])
                        vp_t = vin.tile([128, j_chunks, D], dt_in)
                        nc.sync.dma_start(vp_t[:], vp[h, :, :, :])
                        head_tiles[h] = (qt_t, kt_t, vp_t)
                    return head_tiles[h]

                def emit_qk(h, g, jj):
                    qt_t, kt_t, _ = ensure_head(h)
                    s_t = sp.tile([128, exp_fuse, sq], f32)
                    if "qk" in stages:
                        for u in range(exp_fuse):
                            j = jj * exp_fuse + u
                            for c in range(nsub):
                                nc.tensor.matmul(
                                    s_t[:, u, c * nmm:(c + 1) * nmm],
                                    lhsT=kt_t[:, j * 128:(j + 1) * 128],
                                    rhs=qt_t[:, g * sq + c * nmm:
                                             g * sq + (c + 1) * nmm],
                                )
                    return s_t

                def emit_rest(h, g, jj, s_t):
                    _, _, vp_t = head_tiles[h]
                    if (h, g) not in group_psum:
                        group_psum[(h, g)] = (
                            op.tile([D, sq], f32, name="o_t"),
                            None if sum_fold <= 0 else
                            smp.tile([128, sq], f32, name="sm_t"))
                    o_t, sm_t = group_psum[(h, g)]
                    chunk_no = chunk_counter[0]
                    chunk_counter[0] += 1
                    use_dve = (dve_exp_period and
                               chunk_no % dve_exp_period == dve_exp_phase)
                    if use_dve:
                        # Schraudolph exp on DVE: bitcast(int16(x*A + B)) in
                        # bf16 domain -- one tensor_scalar, ~1.8% rms error,
                        # offloads the ScalarE exp bottleneck
                        pt_i = ptp.tile([128, exp_fuse, sq], mybir.dt.int16,
                                        name="pt")
                        if "exp" in stages:
                            nc.vector.tensor_scalar(
                                pt_i[:], s_t[:], DVE_EXP_A, DVE_EXP_B,
                                op0=mybir.AluOpType.mult,
                                op1=mybir.AluOpType.add)
                        pt_t = pt_i.bitcast(mybir.dt.bfloat16)
                    else:
                        pt_t = ptp.tile([128, exp_fuse, sq], dt_in,
                                        name="pt")
                        if "exp" in stages:
                            nc.scalar.activation(pt_t[:], s_t[:], Exp,
                                                 scale=SOFTMAX_SCALE)
                    if sum_merge and "sums" in stages:
                        # u-merged chain: one [128, exp_fuse*sq]-wide add per
                        # chunk-pair (half the DVE instructions); first
                        # pool_adds adds go to the idle GpSimd/Pool engine
                        if jj == 0:
                            fold_state[(h, g)] = pt_t[:]
                        else:
                            acc = fold_state[(h, g)]
                            t = fsp.tile([128, exp_fuse, sq], dt_in,
                                         name="fs")
                            eng = (nc.gpsimd if jj <= pool_adds
                                   else nc.vector)
                            eng.tensor_add(t[:], acc, pt_t[:])
                            fold_state[(h, g)] = t[:]
                        if (jj + 1) * exp_fuse == j_chunks:
                            acc = fold_state.pop((h, g))
                            mrg = fsp.tile([128, sq], dt_in, name="fsm")
                            nc.vector.tensor_add(mrg[:], acc[:, 0, :],
                                                 acc[:, 1, :])
                            if sm_defer:
                                # emit the ones-matmul later (with the tail)
                                # so it can't block QK/PV in the PE queue
                                # while the DVE chain finishes
                                group_psum[(h, g)] = (o_t, ("acc", mrg[:]))
                            else:
                                if sum_fold == -2:
                                    sm_t = smp.tile([128, sq], f32,
                                                    name="sm_t")
                                    smdst = sm_t[:]
                                else:
                                    sm_t = sp.tile([128, exp_fuse, sq], f32,
                                                   name="sm_t", tag="s_t")
                                    smdst = sm_t[:, 0, :]
                                for c in range(nsub):
                                    cs = slice(c * nmm, (c + 1) * nmm)
                                    nc.tensor.matmul(
                                        smdst[:, cs], lhsT=ones[:],
                                        rhs=mrg[:, cs], start=True, stop=True)
                                group_psum[(h, g)] = (o_t, smdst)
                    for u in range(exp_fuse):
                        j = jj * exp_fuse + u
                        first, last = (j == 0), (j == j_chunks - 1)
                        if "pv" in stages:
                            for c in range(nsub):
                                cs = slice(c * nmm, (c + 1) * nmm)
                                nc.tensor.matmul(
                                    o_t[:, cs], lhsT=vp_t[:, j, :],
                                    rhs=pt_t[:, u, cs],
                                    start=first, stop=last)
                        if "sums" in stages and not sum_merge:
                            if sum_fold <= 0:
                                if j == 0:
                                    fold_state[(h, g)] = pt_t[:, u, :]
                                else:
                                    acc = fold_state[(h, g)]
                                    t = fsp.tile([128, sq], dt_in, name="fs")
                                    nc.vector.tensor_add(t[:], acc,
                                                         pt_t[:, u, :])
                                    fold_state[(h, g)] = t[:]
                                if last and sum_fold == 0:
                                    fin = fold_state.pop((h, g))
                                    sm_t = sp.tile([128, exp_fuse, sq], f32,
                                                   name="sm_t", tag="s_t")
                                    for c in range(nsub):
                                        cs = slice(c * nmm, (c + 1) * nmm)
                                        nc.tensor.matmul(
                                            sm_t[:, 0, cs], lhsT=ones[:],
                                            rhs=fin[:, cs],
                                            start=True, stop=True)
                                    group_psum[(h, g)] = (o_t, sm_t[:, 0, :])
                                elif last and sum_fold == -2:
                                    fin = fold_state.pop((h, g))
                                    sm_t = smp.tile([128, sq], f32,
                                                    name="sm_t")
                                    for c in range(nsub):
                                        cs = slice(c * nmm, (c + 1) * nmm)
                                        nc.tensor.matmul(
                                            sm_t[:, cs], lhsT=ones[:],
                                            rhs=fin[:, cs],
                                            start=True, stop=True)
                                    group_psum[(h, g)] = (o_t, sm_t[:])
                                elif last:
                                    from concourse import bass_isa
                                    smg = fsp.tile([128, sq], f32, name="smg")
                                    nc.gpsimd.partition_all_reduce(
                                        smg[:], fold_state.pop((h, g)),
                                        channels=128,
                                        reduce_op=bass_isa.ReduceOp.add)
                                    group_psum[(h, g)] = (o_t, smg[:])
                            elif sum_fold == 1:
                                for c in range(nsub):
                                    cs = slice(c * nmm, (c + 1) * nmm)
                                    nc.tensor.matmul(
                                        sm_t[:, cs], lhsT=ones[:],
                                        rhs=pt_t[:, u, cs],
                                        start=first, stop=last)
                            else:
                                # Fold pairs/quads of P^T chunks on DVE (bf16
                                # 2x) so the ones-matmul streams fewer columns
                                # on PE.  fp32 PSUM still does the final
                                # accumulation across fold-runs.
                                fl = fold_state.setdefault((h, g), [])
                                fl.append(pt_t[:, u, :])
                                if len(fl) == sum_fold:
                                    work = list(fl)
                                    fl.clear()
                                    while len(work) > 1:
                                        nxt = []
                                        for a, b in zip(work[::2], work[1::2]):
                                            t = fsp.tile([128, sq], dt_in,
                                                         name="fs")
                                            nc.vector.tensor_add(t[:], a, b)
                                            nxt.append(t[:])
                                        work = nxt
                                    run = j // sum_fold
                                    for c in range(nsub):
                                        cs = slice(c * nmm, (c + 1) * nmm)
                                        nc.tensor.matmul(
                                            sm_t[:, cs], lhsT=ones[:],
                                            rhs=work[0][:, cs],
                                            start=(run == 0),
                                            stop=(run == j_chunks //
                                                  sum_fold - 1))
                    return (h, g) if (jj + 1) * exp_fuse == j_chunks else None

                def emit_tail(h, g):
                    has_tail = ("tail" in stages or
                                any(s.startswith("tail_") for s in stages))
                    if not has_tail:
                        group_psum.pop((h, g), None)
                        return
                    o_t, sm_t = group_psum[(h, g)]
                    if isinstance(sm_t, tuple) and sm_t[0] == "acc":
                        mrg = sm_t[1]
                        smt = smp.tile([128, sq], f32, name="sm_t")
                        for c in range(nsub):
                            cs = slice(c * nmm, (c + 1) * nmm)
                            nc.tensor.matmul(smt[:, cs], lhsT=ones[:],
                                             rhs=mrg[:, cs],
                                             start=True, stop=True)
                        sm_t = smt[:]
                    full_tail = "tail" in stages
                    recip_t = rcp.tile([128, sq], f32)
                    if recip_fast:
                        # ~51 ULP, ~5x faster than InstReciprocal; inputs are
                        # sums of positive exps (1e2..1e5), no edge cases
                        nc.vector.reciprocal_approx_fast(recip_t[:], sm_t[:])
                    else:
                        nc.vector.reciprocal(recip_t[:], sm_t[:])
                    if full_tail or "tail_mul" in stages \
                            or "tail_dma" in stages:
                        otn = outsb.tile([D, sq], f32)
                        nc.vector.tensor_mul(otn[:], o_t[:], recip_t[:])
                        if full_tail or "tail_dma" in stages:
                            nc.sync.dma_start(
                                outT[h, :, g * sq:(g + 1) * sq], otn[:])
                    del group_psum[(h, g)]

                if prefetch:
                    for h in range(hl):
                        ensure_head(h)
                if interleave_groups and groups >= 2:
                    chunk_list = [(h, p + g, jj) for h in range(hl)
                                  for p in range(0, groups, 2)
                                  for jj in range(jj_units)
                                  for g in range(min(2, groups - p))]
                else:
                    chunk_list = [(h, g, jj) for h in range(hl)
                                  for g in range(groups)
                                  for jj in range(jj_units)]
                pending = []
                tails = []  # [(h, g, countdown)] deferred group tails

                def step_tails(flush=False):
                    while tails and (flush or tails[0][2] <= 0):
                        th, tg, _ = tails.pop(0)
                        emit_tail(th, tg)
                    for i_ in range(len(tails)):
                        th, tg, cd = tails[i_]
                        tails[i_] = (th, tg, cd - 1)

                for ch in chunk_list:
                    pending.append((ch, emit_qk(*ch)))
                    if len(pending) > lookahead:
                        (h, g, j), s_t = pending.pop(0)
                        done = emit_rest(h, g, j, s_t)
                        if done:
                            tails.append((*done, tail_defer))
                        step_tails()
                while pending:
                    (h, g, j), s_t = pending.pop(0)
                    done = emit_rest(h, g, j, s_t)
                    if done:
                        tails.append((*done, tail_defer))
                    step_tails()
                step_tails(flush=True)

            def body_band(_it=None):
                """Band mode: one s/pt tile holds band_w groups' columns for
                the SAME k-chunk j, so each kt[j]/vp[j] weight load serves
                band_w matmuls (PE per exp-unit drops below the ScalarE exp
                time, breaking the PE<->ScalarE lockstep).  Iteration order:
                (head, band, j)."""
                bw = band_w
                nb = groups // bw
                assert groups % bw == 0
                head_tiles = {}
                o_tiles = {}
                band_acc = {}
                group_psum = {}
                unit_counter = [0]

                def ensure_head(h):
                    if h not in head_tiles:
                        qt_t = qin.tile([D, s], dt_in)
                        nc.sync.dma_start(qt_t[:], qt[h, :, :])
                        kt_t = kin.tile([D, s], dt_in)
                        nc.sync.dma_start(kt_t[:], kt[h, :, :])
                        vp_t = vin.tile([128, j_chunks, D], dt_in)
                        nc.sync.dma_start(vp_t[:], vp[h, :, :, :])
                        head_tiles[h] = (qt_t, kt_t, vp_t)
                    return head_tiles[h]

                def emit_qk_band(h, band, j):
                    qt_t, kt_t, _ = ensure_head(h)
                    s_bt = sp.tile([128, bw, sq], f32)
                    if "qk" in stages:
                        for gb in range(bw):
                            g = band * bw + gb
                            for c in range(nsub):
                                cs = slice(c * nmm, (c + 1) * nmm)
                                nc.tensor.matmul(
                                    s_bt[:, gb, cs],
                                    lhsT=kt_t[:, j * 128:(j + 1) * 128],
                                    rhs=qt_t[:, g * sq + c * nmm:
                                             g * sq + (c + 1) * nmm])
                    return s_bt

                def emit_rest_band(h, band, j, s_bt):
                    _, _, vp_t = head_tiles[h]
                    uno = unit_counter[0]
                    unit_counter[0] += 1
                    use_dve = (dve_exp_period and
                               uno % dve_exp_period == dve_exp_phase)
                    if use_dve:
                        pt_i = ptp.tile([128, bw, sq], mybir.dt.int16,
                                        name="pt")
                        if "exp" in stages:
                            nc.vector.tensor_scalar(
                                pt_i[:], s_bt[:], DVE_EXP_A, DVE_EXP_B,
                                op0=mybir.AluOpType.mult,
                                op1=mybir.AluOpType.add)
                        pt_bt = pt_i.bitcast(mybir.dt.bfloat16)
                    else:
                        pt_bt = ptp.tile([128, bw, sq], dt_in, name="pt")
                        if "exp" in stages:
                            nc.scalar.activation(pt_bt[:], s_bt[:], Exp,
                                                 scale=SOFTMAX_SCALE)
                    first, last = (j == 0), (j == j_chunks - 1)
                    if "pv" in stages:
                        for gb in range(bw):
                            g = band * bw + gb
                            if (h, g) not in o_tiles:
                                o_tiles[(h, g)] = op.tile([D, sq], f32,
                                                          name="o_t")
                            for c in range(nsub):
                                cs = slice(c * nmm, (c + 1) * nmm)
                                nc.tensor.matmul(
                                    o_tiles[(h, g)][:, cs],
                                    lhsT=vp_t[:, j, :],
                                    rhs=pt_bt[:, gb, cs],
                                    start=first, stop=last)
                    if "sums" in stages:
                        if first:
                            band_acc[(h, band)] = pt_bt[:]
                        else:
                            t = fsp.tile([128, bw, sq], dt_in, name="fs")
                            nc.vector.tensor_add(t[:], band_acc[(h, band)],
                                                 pt_bt[:])
                            band_acc[(h, band)] = t[:]
                        if last:
                            acc = band_acc.pop((h, band))
                            if sum_fold == -2:
                                sm_t = smp.tile([128, bw, sq], f32,
                                                name="sm_t")
                                sm_aps = [sm_t[:, gb, :] for gb in range(bw)]
                            else:
                                sm_t = sp.tile([128, bw, sq], f32,
                                               name="sm_t", tag="s_t")
                                sm_aps = [sm_t[:, gb, :] for gb in range(bw)]
                            for gb in range(bw):
                                g = band * bw + gb
                                for c in range(nsub):
                                    cs = slice(c * nmm, (c + 1) * nmm)
                                    nc.tensor.matmul(
                                        sm_aps[gb][:, cs], lhsT=ones[:],
                                        rhs=acc[:, gb, cs],
                                        start=True, stop=True)
                                group_psum[(h, g)] = (
                                    o_tiles.pop((h, g), None), sm_aps[gb])
                    return ([(h, band * bw + gb) for gb in range(bw)]
                            if last else None)

                def emit_tail_band(h, g):
                    if "tail" not in stages:
                        group_psum.pop((h, g), None)
                        return
                    o_t, sm_ap = group_psum[(h, g)]
                    recip_t = rcp.tile([128, sq], f32)
                    if recip_fast:
                        nc.vector.reciprocal_approx_fast(recip_t[:], sm_ap)
                    else:
                        nc.vector.reciprocal(recip_t[:], sm_ap)
                    otn = outsb.tile([D, sq], f32)
                    nc.vector.tensor_mul(otn[:], o_t[:], recip_t[:])
                    nc.sync.dma_start(outT[h, :, g * sq:(g + 1) * sq],
                                      otn[:])
                    del group_psum[(h, g)]

                if prefetch:
                    for h in range(hl):
                        ensure_head(h)
                unit_list = [(h, b, j) for h in range(hl)
                             for b in range(nb) for j in range(j_chunks)]
                pending = []
                tails = []

                def step_tails(flush=False):
                    while tails and (flush or tails[0][2] <= 0):
                        th, tg, _ = tails.pop(0)
                        emit_tail_band(th, tg)
                    for i_ in range(len(tails)):
                        th, tg, cd = tails[i_]
                        tails[i_] = (th, tg, cd - 1)

                for unit in unit_list:
                    pending.append((unit, emit_qk_band(*unit)))
                    if len(pending) > lookahead:
                        u, s_bt = pending.pop(0)
                        done = emit_rest_band(*u, s_bt)
                        if done:
                            tails.extend((hh, gg, tail_defer)
                                         for (hh, gg) in done)
                        step_tails()
                while pending:
                    u, s_bt = pending.pop(0)
                    done = emit_rest_band(*u, s_bt)
                    if done:
                        tails.extend((hh, gg, tail_defer)
                                     for (hh, gg) in done)
                    step_tails()
                step_tails(flush=True)

            if band_mode:
                body = body_band

            if repeat == 1:
                body()
            elif repeat < 0:
                # negative: Python-unrolled (for TimelineSim, which can't
                # follow For_i register branches without an executor)
                for _r in range(-repeat):
                    body(_r)
            else:
                with tc.For_i(0, repeat, 1) as _i:
                    body(_i)

            if bench_internal:
                dtile = singles.tile([2, 1], mybir.dt.float32, name="dumt")
                nc.sync.dma_start(dtile[:], dummy_in[:])
                nc.sync.dma_start(dummy_out[:], dtile[:])

    nc.compile()
    return nc, np_in


def shard_inputs(q, k, v, s=S, hl=HL, ncores=NCORES, np_in=BF16):
    """Host-side layout prep: per-core per-head transposed views, cast."""
    nheads = ncores * hl
    j_chunks = s // 128
    # [s,b,n,d] -> [b,n,d,s] -> [H, d, s]
    qt = np.ascontiguousarray(q.transpose(1, 2, 3, 0)).reshape(nheads, D, s)
    kt = np.ascontiguousarray(k.transpose(1, 2, 3, 0)).reshape(nheads, D, s)
    # [s,b,n,d] -> [b,n,s,d] -> [H, J, 128, d] -> [H, 128, J, d]
    vpm = (v.transpose(1, 2, 0, 3)
            .reshape(nheads, j_chunks, 128, D)
            .transpose(0, 2, 1, 3))
    qt = qt.astype(np_in)
    kt = kt.astype(np_in)
    vpm = np.ascontiguousarray(vpm).astype(np_in)
    in_maps = []
    for c in range(ncores):
        sl = slice(c * hl, (c + 1) * hl)
        in_maps.append({
            "qt": np.ascontiguousarray(qt[sl]),
            "kt": np.ascontiguousarray(kt[sl]),
            "vp": np.ascontiguousarray(vpm[sl]),
        })
    return in_maps


def gather_output(results, s=S, hl=HL, ncores=NCORES):
    """[{outT: [hl, D, s]}] per core -> full [s, B, NH*D] fp32."""
    outT = np.stack([np.asarray(r["outT"]) for r in results])  # [C, hl, D, s]
    out_heads = outT.reshape(ncores * hl, D, s)                # [H, D, s]
    out = out_heads.transpose(2, 0, 1)                         # [s, H, D]
    return np.ascontiguousarray(out).reshape(s, B, NH * D)


_CACHE = {}

# Best measured configuration (~145 us/core steady state, vs ~138 us ScalarE
# exp roofline):
#   recip_fast     -- InstReciprocal is ~5x a normal DVE op and was blocking
#                     the in-order DVE queue (+45 us); approx_fast fixes it.
#   sum_merge      -- one u-merged [128,1024] chain add per chunk pair
#                     (half the DVE sums instructions).
#   sum_fold=-2    -- sums ones-matmul lands in a dedicated double-buffered
#                     PSUM bank (sm_bufs=2) instead of borrowing a QK<->exp
#                     s-pool slot; PSUM: s 2x2 + sm 2 + o 2 = 8 banks.
#   in_bufs=3      -- triple-buffered per-head input DMAs.
#   sm_defer+tail_defer=2 -- the sums ones-matmul and the whole tail are
#                     emitted two chunks after the group completes, so the
#                     in-order PE queue never blocks on the DVE chain.
BEST_CFG = dict(sq=512, nmm=512, mm_dtype="bf16", lookahead=2, s_bufs=2,
                o_bufs=2, sm_bufs=2, exp_fuse=2, sum_fold=-2, sum_merge=True,
                in_bufs=3, pt_bufs=8, tail_bufs=3, fs_bufs=5,
                recip_fast=True, sm_defer=True, tail_defer=2)


def _get_program(**cfg):
    key = tuple(sorted(cfg.items()))
    if key not in _CACHE:
        _CACHE[key] = build_program(**cfg)
    return _CACHE[key]


def run(q, k, v, trace=False, **cfg):
    """Run on the 8 NeuronCores; returns (out, BassKernelResults)."""
    from concourse.bass_utils import run_bass_kernel_spmd

    full_cfg = {**BEST_CFG, **cfg}
    nc, np_in = _get_program(**full_cfg)
    in_maps = shard_inputs(q, k, v, np_in=np_in)
    res = run_bass_kernel_spmd(nc, in_maps, core_ids=list(range(NCORES)),
                               trace=trace)
    return gather_output(res.results), res


def kernel(q, k, v):
    q = np.asarray(q, dtype=np.float32)
    k = np.asarray(k, dtype=np.float32)
    v = np.asarray(v, dtype=np.float32)
    out, _ = run(q, k, v)
    return out

